# revision 15
# baseline (speedup 1.0000x reference)
"""Point-cloud-to-mesh loss on 8 Trainium2 cores.

Math: the reference loss only needs, per query point, the minimum squared
distance to any mesh triangle (the residual ``p - cp`` has squared norm equal
to the min point-triangle distance^2).  For a triangle (a,b,c) with edges
ab, ac, bc the min distance^2 decomposes into four candidates:

  e_edge = E_unc + relu(max(h1, h2))^2   for each of the 3 edges, where
           E_unc = f - d^2/L  (unclamped line distance^2),
           h1 = (d - L)/sqrt(L), h2 = -d/sqrt(L)  (clamp overshoots)
  r_in   = max(f - (vb*d1 + vc*d2)/det, 0) + BIG*(relu(-va/det)+...)
           (plane distance^2, masked unless the projection is interior)

Every E_unc / h / penalty argument is an affine or quadratic polynomial in p,
so with point features phi(p) = [1,x,y,z,x^2,y^2,z^2,xy,xz,yz] all 13
quantities per (point, face) pair are a single K=10 matmul.  The TensorE
produces them; ScalarE does relu/square; VectorE does max/add/min and the
fused (min, min-reduce, running-min) via tensor_tensor_reduce.

Sharding: 8 cores = 2 batches x 4 point-chunks of 2048 points; every core
holds all 8192 faces of its batch.  Each core returns per-point min-d^2;
the host does the final sqrt / height normalization / mean.
"""
import math

import numpy as np

import concourse.bass as bass
import concourse.bacc as bacc
import concourse.mybir as mybir
import concourse.tile as tile
from concourse.bass_utils import run_bass_kernel_spmd

# Problem shape (hardcoded per spec)
B = 2
P_FULL = 8192           # query points per batch
V = 4098
F = 8192                # faces per batch
N_CORES = 8
P_CORE = (B * P_FULL) // N_CORES     # 2048 points per core
N_PT = P_CORE // 128                 # 16 point tiles
FT = 512                             # faces per chunk
N_FC = F // FT                       # 16 face chunks
K = 10                               # monomial features
NOUT = 13                            # per-face matmul outputs

EPS = 1e-12
BIGP = 1e6
ACC_INIT = 1e30

FP32 = mybir.dt.float32
AX = mybir.AxisListType
OP = mybir.AluOpType
AF = mybir.ActivationFunctionType

_NC_CACHE = None


def build_nc(repeats=1):
    """Build the SPMD Bass program (same program on all 8 cores).
    repeats>1 re-runs the whole face sweep (idempotent min) for timing."""
    nc = bacc.Bacc("TRN2")
    phi_d = nc.dram_tensor("phi", [K, P_CORE], FP32, kind="ExternalInput")
    w_d = nc.dram_tensor("w", [N_FC, K, NOUT, FT], FP32, kind="ExternalInput")
    out_d = nc.dram_tensor("out", [128, N_PT], FP32, kind="ExternalOutput")

    with tile.TileContext(nc) as tc:
        with (
            tc.tile_pool(name="persist", bufs=1) as persist,
            tc.tile_pool(name="wpool", bufs=2) as wpool,
            tc.tile_pool(name="sb", bufs=3) as sb,
            tc.tile_pool(name="psumE", bufs=5, space="PSUM") as psumE,
            tc.tile_pool(name="psumHP", bufs=3, space="PSUM") as psumHP,
        ):
            phi_s = persist.tile([K, P_CORE], FP32)
            nc.gpsimd.dma_start(out=phi_s, in_=phi_d[:, :])

            acc = [persist.tile([128, N_PT], FP32, tag=f"acc{i}", name=f"acc{i}")
                   for i in range(2)]
            nc.vector.memset(acc[0], ACC_INIT)

            for it in range(repeats * N_FC):
                fc = it % N_FC
                w_s = wpool.tile([K, NOUT, FT], FP32, tag="w")
                nc.gpsimd.dma_start(out=w_s, in_=w_d[fc])
                a_prev = acc[it % 2]
                a_cur = acc[(it + 1) % 2]
                for pt in range(N_PT):
                    lhsT = phi_s[:, pt * 128:(pt + 1) * 128]

                    def mm(j, pool, tag):
                        ps = pool.tile([128, FT], FP32, tag=tag, name=f"mm{j}")
                        nc.tensor.matmul(ps, lhsT, w_s[:, j, :])
                        return ps

                    # --- edges ---
                    edges = []
                    for (je, jh1, jh2, enm) in ((0, 7, 8, "ab"), (1, 9, 10, "ac"),
                                                (2, 11, 12, "bc")):
                        E_ps = mm(je, psumE, "E")
                        h1_ps = mm(jh1, psumHP, "HP")
                        h2_ps = mm(jh2, psumHP, "HP")
                        a2r = sb.tile([128, FT], FP32, tag="a2r")
                        nc.scalar.activation(a2r, h2_ps, AF.Relu)
                        z = sb.tile([128, FT], FP32, tag="z")
                        nc.vector.tensor_tensor(z, h1_ps, a2r, OP.max)
                        zz = sb.tile([128, FT], FP32, tag="zz")
                        nc.scalar.activation(zz, z, AF.Square)
                        e = sb.tile([128, FT], FP32, tag=f"e_{enm}")
                        nc.vector.tensor_tensor(e, E_ps, zz, OP.add)
                        edges.append(e)

                    # --- interior ---
                    rin_ps = mm(3, psumE, "E")
                    pens = []
                    for jp in (4, 5, 6):
                        p_ps = mm(jp, psumHP, "HP")
                        pr = sb.tile([128, FT], FP32, tag=f"pen{jp}")
                        nc.scalar.activation(pr, p_ps, AF.Relu)
                        pens.append(pr)
                    pp = sb.tile([128, FT], FP32, tag="pp")
                    nc.vector.tensor_tensor(pp, pens[0], pens[1], OP.add)
                    pp2 = sb.tile([128, FT], FP32, tag="pp2")
                    nc.vector.tensor_tensor(pp2, pp, pens[2], OP.add)
                    r1 = sb.tile([128, FT], FP32, tag="r1")
                    nc.vector.scalar_tensor_tensor(r1, rin_ps, 0.0, pp2,
                                                   OP.max, OP.add)

                    # --- min tree + reduce & running min ---
                    m1 = sb.tile([128, FT], FP32, tag="m1")
                    nc.vector.tensor_tensor(m1, edges[0], edges[1], OP.min)
                    m2 = sb.tile([128, FT], FP32, tag="m2")
                    nc.vector.tensor_tensor(m2, m1, edges[2], OP.min)
                    m3 = sb.tile([128, FT], FP32, tag="m3")
                    nc.vector.tensor_tensor(m3, m2, r1, OP.min)
                    red = sb.tile([128, 1], FP32, tag="red")
                    nc.vector.tensor_reduce(red, m3, AX.X, OP.min)
                    nc.vector.tensor_tensor(a_cur[:, pt:pt + 1],
                                            a_prev[:, pt:pt + 1], red, OP.min)

            nc.sync.dma_start(out=out_d[:, :], in_=acc[(repeats * N_FC) % 2])
    nc.compile()
    return nc


def get_nc():
    global _NC_CACHE
    if _NC_CACHE is None:
        _NC_CACHE = build_nc()
    return _NC_CACHE


# ---------------- host-side weight/feature prep ----------------

def _face_weights(a, b, c):
    """[10, 13, F] fp32 monomial weights for all candidate quantities.
    Monomial basis: [1, x, y, z, x^2, y^2, z^2, xy, xz, yz]."""
    a = a.astype(np.float64)
    b = b.astype(np.float64)
    c = c.astype(np.float64)
    ab, ac, bc = b - a, c - a, c - b
    aa = (ab * ab).sum(-1)
    bb = (ab * ac).sum(-1)
    cc = (ac * ac).sum(-1)
    ll = (bc * bc).sum(-1)
    det = aa * cc - bb * bb
    inv_aa = 1.0 / np.maximum(aa, EPS)
    inv_cc = 1.0 / np.maximum(cc, EPS)
    inv_ll = 1.0 / np.maximum(ll, EPS)
    inv_det = 1.0 / np.maximum(det, EPS)
    nF = a.shape[0]

    def affine(const, lin):
        w = np.zeros((nF, 10))
        w[:, 0] = const
        w[:, 1:4] = lin
        return w

    def prod(c1, l1, c2, l2):
        w = np.zeros((nF, 10))
        w[:, 0] = c1 * c2
        w[:, 1:4] = c1[:, None] * l2 + c2[:, None] * l1
        w[:, 4] = l1[:, 0] * l2[:, 0]
        w[:, 5] = l1[:, 1] * l2[:, 1]
        w[:, 6] = l1[:, 2] * l2[:, 2]
        w[:, 7] = l1[:, 0] * l2[:, 1] + l1[:, 1] * l2[:, 0]
        w[:, 8] = l1[:, 0] * l2[:, 2] + l1[:, 2] * l2[:, 0]
        w[:, 9] = l1[:, 1] * l2[:, 2] + l1[:, 2] * l2[:, 1]
        return w

    def sqdist(v):
        w = np.zeros((nF, 10))
        w[:, 0] = (v * v).sum(-1)
        w[:, 1:4] = -2.0 * v
        w[:, 4:7] = 1.0
        return w

    d1c, d1l = -(ab * a).sum(-1), ab
    d2c, d2l = -(ac * a).sum(-1), ac
    uc, ul = -(bc * b).sum(-1), bc
    f_w = sqdist(a)
    rb_w = sqdist(b)
    vbc = d1c * cc - d2c * bb
    vbl = d1l * cc[:, None] - d2l * bb[:, None]
    vcc = d2c * aa - d1c * bb
    vcl = d2l * aa[:, None] - d1l * bb[:, None]
    vac = det - vbc - vcc
    val = -vbl - vcl
    r_aa, r_cc, r_ll = np.sqrt(inv_aa), np.sqrt(inv_cc), np.sqrt(inv_ll)

    W = np.zeros((10, NOUT, nF))
    W[:, 0] = (f_w - prod(d1c, d1l, d1c, d1l) * inv_aa[:, None]).T
    W[:, 1] = (f_w - prod(d2c, d2l, d2c, d2l) * inv_cc[:, None]).T
    W[:, 2] = (rb_w - prod(uc, ul, uc, ul) * inv_ll[:, None]).T
    W[:, 3] = (f_w - (prod(vbc, vbl, d1c, d1l)
                      + prod(vcc, vcl, d2c, d2l)) * inv_det[:, None]).T
    W[:, 4] = (affine(vac, val) * (-BIGP * inv_det)[:, None]).T
    W[:, 5] = (affine(vbc, vbl) * (-BIGP * inv_det)[:, None]).T
    W[:, 6] = (affine(vcc, vcl) * (-BIGP * inv_det)[:, None]).T
    W[:, 7] = (affine(d1c - aa, d1l) * r_aa[:, None]).T
    W[:, 8] = (affine(d1c, d1l) * (-r_aa)[:, None]).T
    W[:, 9] = (affine(d2c - cc, d2l) * r_cc[:, None]).T
    W[:, 10] = (affine(d2c, d2l) * (-r_cc)[:, None]).T
    W[:, 11] = (affine(uc - ll, ul) * r_ll[:, None]).T
    W[:, 12] = (affine(uc, ul) * (-r_ll)[:, None]).T
    return np.ascontiguousarray(W.astype(np.float32))


def _features(points):
    p = points.astype(np.float32)
    x, y, z = p[:, 0], p[:, 1], p[:, 2]
    return np.ascontiguousarray(np.stack(
        [np.ones_like(x), x, y, z, x * x, y * y, z * z, x * y, x * z, y * z],
        axis=0))


def kernel(body_verts, mesh_verts, faces):
    body_verts = np.asarray(body_verts, dtype=np.float32)
    mesh_verts = np.asarray(mesh_verts, dtype=np.float32)
    faces = np.asarray(faces, dtype=np.int32)

    nc = get_nc()

    in_maps = []
    for core in range(N_CORES):
        bi = core // (N_CORES // B)
        chunk = core % (N_CORES // B)
        a = mesh_verts[bi][faces[bi][:, 0]]
        b = mesh_verts[bi][faces[bi][:, 1]]
        c = mesh_verts[bi][faces[bi][:, 2]]
        if chunk == 0:
            W = _face_weights(a, b, c)            # [10, 13, F]
            # chunk faces: [N_FC, K, NOUT, FT]
            Wc = np.ascontiguousarray(
                W.reshape(K, NOUT, N_FC, FT).transpose(2, 0, 1, 3))
            in_maps_batch_w = Wc
        pts = body_verts[bi][chunk * P_CORE:(chunk + 1) * P_CORE]
        in_maps.append({"phi": _features(pts), "w": in_maps_batch_w})

    res = run_bass_kernel_spmd(nc, in_maps, core_ids=list(range(N_CORES)))

    losses = []
    for bi in range(B):
        md = np.concatenate([
            np.asarray(res.results[bi * (N_CORES // B) + ch]["out"])
            .T.reshape(-1)
            for ch in range(N_CORES // B)
        ])
        height = (mesh_verts[bi][:, 1].max() - mesh_verts[bi][:, 1].min())
        losses.append(math.sqrt(float(md.astype(np.float64).sum())) / float(height))
    return np.float32(sum(losses) / B)


# revision 20
# speedup vs baseline: 1.4312x; 1.4312x over previous
"""Point-cloud-to-mesh loss on 8 Trainium2 cores.

Math: the reference loss only needs, per query point, the minimum squared
distance to any mesh triangle (the residual ``p - cp`` has squared norm equal
to the min point-triangle distance^2).  For a triangle (a,b,c) with edges
ab, ac, bc the min distance^2 decomposes into four candidates:

  e_edge = E_unc + relu(max(h1, h2))^2   for each of the 3 edges, where
           E_unc = f - d^2/L  (unclamped line distance^2),
           h1 = (d - L)/sqrt(L), h2 = -d/sqrt(L)  (clamp overshoots)
  r_in   = max(f - (vb*d1 + vc*d2)/det, 0) + BIG*(relu(-va/det)+...)
           (plane distance^2, masked unless the projection is interior)

Every E_unc / h / penalty argument is an affine or quadratic polynomial in p,
so with point features phi(p) = [1,x,y,z,x^2,y^2,z^2,xy,xz,yz] all 13
quantities per (point, face) pair are a single K=10 matmul.  The TensorE
produces them; ScalarE does relu/square; VectorE does max/add/min and the
fused (min, min-reduce, running-min) via tensor_tensor_reduce.

Sharding: 8 cores = 2 batches x 4 point-chunks of 2048 points; every core
holds all 8192 faces of its batch.  Each core returns per-point min-d^2;
the host does the final sqrt / height normalization / mean.
"""
import math

import numpy as np

import concourse.bass as bass
import concourse.bacc as bacc
import concourse.mybir as mybir
import concourse.tile as tile
from concourse.bass_utils import run_bass_kernel_spmd

# Problem shape (hardcoded per spec)
B = 2
P_FULL = 8192           # query points per batch
V = 4098
F = 8192                # faces per batch
N_CORES = 8
P_CORE = (B * P_FULL) // N_CORES     # 2048 points per core
N_PT = P_CORE // 128                 # 16 point tiles
FT = 512                             # faces per chunk
N_FC = F // FT                       # 16 face chunks
K = 10                               # monomial features
NOUT = 13                            # per-face matmul outputs

EPS = 1e-12
BIGP = 1e6
ACC_INIT = 1e30

FP32 = mybir.dt.float32
AX = mybir.AxisListType
OP = mybir.AluOpType
AF = mybir.ActivationFunctionType

_NC_CACHE = None


def build_nc(repeats=1, gp_offload=False):
    """Build the SPMD Bass program (same program on all 8 cores).
    repeats>1 re-runs the whole face sweep (idempotent min) for timing."""
    nc = bacc.Bacc("TRN2")
    phi_d = nc.dram_tensor("phi", [K, P_CORE], FP32, kind="ExternalInput")
    w_d = nc.dram_tensor("w", [N_FC, K, NOUT, FT], FP32, kind="ExternalInput")
    out_d = nc.dram_tensor("out", [128, N_PT], FP32, kind="ExternalOutput")

    with tile.TileContext(nc) as tc:
        with (
            tc.tile_pool(name="persist", bufs=1) as persist,
            tc.tile_pool(name="wpool", bufs=2) as wpool,
            tc.tile_pool(name="sb", bufs=3) as sb,
            tc.tile_pool(name="psumE3", bufs=1, space="PSUM") as psumE3,
            tc.tile_pool(name="psumR", bufs=2, space="PSUM") as psumR,
            tc.tile_pool(name="psumHP", bufs=3, space="PSUM") as psumHP,
        ):
            # phi replicated at partition offsets 0/32/64/96 so matmuls can
            # cycle the 4 PE row-groups concurrently (K=10 << 128).
            phi_s = persist.tile([128, P_CORE], FP32)
            for g in range(4):
                nc.gpsimd.dma_start(out=phi_s[32 * g:32 * g + K, :],
                                    in_=phi_d[:, :])

            acc = [persist.tile([128, N_PT], FP32, tag=f"acc{i}", name=f"acc{i}")
                   for i in range(2)]
            nc.vector.memset(acc[0], ACC_INIT)

            for it in range(repeats * N_FC):
                fc = it % N_FC
                w_s = wpool.tile([128, NOUT, FT], FP32, tag="w")
                for g in range(4):
                    nc.gpsimd.dma_start(out=w_s[32 * g:32 * g + K, :, :],
                                        in_=w_d[fc])
                a_prev = acc[it % 2]
                a_cur = acc[(it + 1) % 2]
                for pt in range(N_PT):
                    mm_ctr = [0]

                    def mm(j, pool, tag, out=None):
                        if out is None:
                            out = pool.tile([128, FT], FP32, tag=tag,
                                            name=f"mm{j}")
                        g = mm_ctr[0] % 4
                        mm_ctr[0] += 1
                        o = 32 * g
                        nc.tensor.matmul(
                            out, phi_s[o:o + K, pt * 128:(pt + 1) * 128],
                            w_s[o:o + K, j, :], tile_position=(o, 0))
                        return out

                    # --- edges: E3 psum [128,3,FT]; overshoots -> ZZ3 sbuf ---
                    E3 = psumE3.tile([128, 3, FT], FP32, tag="E3", name="E3")
                    ZZ3 = sb.tile([128, 3, FT], FP32, tag="ZZ3")
                    for i, (je, jh1, jh2) in enumerate(
                            ((0, 7, 8), (1, 9, 10), (2, 11, 12))):
                        mm(je, None, None, out=E3[:, i, :])
                        h1_ps = mm(jh1, psumHP, "HP")
                        h2_ps = mm(jh2, psumHP, "HP")
                        a2r = sb.tile([128, FT], FP32, tag="a2r")
                        nc.scalar.activation(a2r, h2_ps, AF.Relu)
                        z = sb.tile([128, FT], FP32, tag="z")
                        nc.vector.tensor_tensor(z, h1_ps, a2r, OP.max)
                        nc.scalar.activation(ZZ3[:, i, :], z, AF.Square)

                    # --- interior: pens ---
                    rin_ps = mm(3, psumR, "RIN")
                    pens = []
                    for jp in (4, 5, 6):
                        p_ps = mm(jp, psumHP, "HP")
                        pr = sb.tile([128, FT], FP32, tag=f"pen{jp}")
                        nc.scalar.activation(pr, p_ps, AF.Relu)
                        pens.append(pr)
                    pp = sb.tile([128, FT], FP32, tag="pp")
                    pp2 = sb.tile([128, FT], FP32, tag="pp2")
                    eng = nc.gpsimd if gp_offload else nc.vector
                    eng.tensor_tensor(pp, pens[0], pens[1], OP.add)
                    eng.tensor_tensor(pp2, pp, pens[2], OP.add)

                    # --- candidates tile e4 [128,4,FT]: 3 edges + interior ---
                    e4 = sb.tile([128, 4, FT], FP32, tag="e4")
                    nc.vector.tensor_tensor(e4[:, 0:3, :], E3, ZZ3, OP.add)
                    nc.vector.scalar_tensor_tensor(e4[:, 3, :], rin_ps, 0.0,
                                                   pp2, OP.max, OP.add)

                    # --- one fused min-reduce over all 4*FT candidates ---
                    red = sb.tile([128, 1], FP32, tag="red")
                    nc.vector.tensor_reduce(red, e4, AX.XY, OP.min)
                    nc.vector.tensor_tensor(a_cur[:, pt:pt + 1],
                                            a_prev[:, pt:pt + 1], red, OP.min)

            nc.sync.dma_start(out=out_d[:, :], in_=acc[(repeats * N_FC) % 2])
    nc.compile()
    return nc


def get_nc():
    global _NC_CACHE
    if _NC_CACHE is None:
        _NC_CACHE = build_nc()
    return _NC_CACHE


# ---------------- host-side weight/feature prep ----------------

def _face_weights(a, b, c):
    """[10, 13, F] fp32 monomial weights for all candidate quantities.
    Monomial basis: [1, x, y, z, x^2, y^2, z^2, xy, xz, yz]."""
    a = a.astype(np.float64)
    b = b.astype(np.float64)
    c = c.astype(np.float64)
    ab, ac, bc = b - a, c - a, c - b
    aa = (ab * ab).sum(-1)
    bb = (ab * ac).sum(-1)
    cc = (ac * ac).sum(-1)
    ll = (bc * bc).sum(-1)
    det = aa * cc - bb * bb
    inv_aa = 1.0 / np.maximum(aa, EPS)
    inv_cc = 1.0 / np.maximum(cc, EPS)
    inv_ll = 1.0 / np.maximum(ll, EPS)
    inv_det = 1.0 / np.maximum(det, EPS)
    nF = a.shape[0]

    def affine(const, lin):
        w = np.zeros((nF, 10))
        w[:, 0] = const
        w[:, 1:4] = lin
        return w

    def prod(c1, l1, c2, l2):
        w = np.zeros((nF, 10))
        w[:, 0] = c1 * c2
        w[:, 1:4] = c1[:, None] * l2 + c2[:, None] * l1
        w[:, 4] = l1[:, 0] * l2[:, 0]
        w[:, 5] = l1[:, 1] * l2[:, 1]
        w[:, 6] = l1[:, 2] * l2[:, 2]
        w[:, 7] = l1[:, 0] * l2[:, 1] + l1[:, 1] * l2[:, 0]
        w[:, 8] = l1[:, 0] * l2[:, 2] + l1[:, 2] * l2[:, 0]
        w[:, 9] = l1[:, 1] * l2[:, 2] + l1[:, 2] * l2[:, 1]
        return w

    def sqdist(v):
        w = np.zeros((nF, 10))
        w[:, 0] = (v * v).sum(-1)
        w[:, 1:4] = -2.0 * v
        w[:, 4:7] = 1.0
        return w

    d1c, d1l = -(ab * a).sum(-1), ab
    d2c, d2l = -(ac * a).sum(-1), ac
    uc, ul = -(bc * b).sum(-1), bc
    f_w = sqdist(a)
    rb_w = sqdist(b)
    vbc = d1c * cc - d2c * bb
    vbl = d1l * cc[:, None] - d2l * bb[:, None]
    vcc = d2c * aa - d1c * bb
    vcl = d2l * aa[:, None] - d1l * bb[:, None]
    vac = det - vbc - vcc
    val = -vbl - vcl
    r_aa, r_cc, r_ll = np.sqrt(inv_aa), np.sqrt(inv_cc), np.sqrt(inv_ll)

    W = np.zeros((10, NOUT, nF))
    W[:, 0] = (f_w - prod(d1c, d1l, d1c, d1l) * inv_aa[:, None]).T
    W[:, 1] = (f_w - prod(d2c, d2l, d2c, d2l) * inv_cc[:, None]).T
    W[:, 2] = (rb_w - prod(uc, ul, uc, ul) * inv_ll[:, None]).T
    W[:, 3] = (f_w - (prod(vbc, vbl, d1c, d1l)
                      + prod(vcc, vcl, d2c, d2l)) * inv_det[:, None]).T
    W[:, 4] = (affine(vac, val) * (-BIGP * inv_det)[:, None]).T
    W[:, 5] = (affine(vbc, vbl) * (-BIGP * inv_det)[:, None]).T
    W[:, 6] = (affine(vcc, vcl) * (-BIGP * inv_det)[:, None]).T
    W[:, 7] = (affine(d1c - aa, d1l) * r_aa[:, None]).T
    W[:, 8] = (affine(d1c, d1l) * (-r_aa)[:, None]).T
    W[:, 9] = (affine(d2c - cc, d2l) * r_cc[:, None]).T
    W[:, 10] = (affine(d2c, d2l) * (-r_cc)[:, None]).T
    W[:, 11] = (affine(uc - ll, ul) * r_ll[:, None]).T
    W[:, 12] = (affine(uc, ul) * (-r_ll)[:, None]).T
    return np.ascontiguousarray(W.astype(np.float32))


def _features(points):
    p = points.astype(np.float32)
    x, y, z = p[:, 0], p[:, 1], p[:, 2]
    return np.ascontiguousarray(np.stack(
        [np.ones_like(x), x, y, z, x * x, y * y, z * z, x * y, x * z, y * z],
        axis=0))


def kernel(body_verts, mesh_verts, faces):
    body_verts = np.asarray(body_verts, dtype=np.float32)
    mesh_verts = np.asarray(mesh_verts, dtype=np.float32)
    faces = np.asarray(faces, dtype=np.int32)

    nc = get_nc()

    in_maps = []
    for core in range(N_CORES):
        bi = core // (N_CORES // B)
        chunk = core % (N_CORES // B)
        a = mesh_verts[bi][faces[bi][:, 0]]
        b = mesh_verts[bi][faces[bi][:, 1]]
        c = mesh_verts[bi][faces[bi][:, 2]]
        if chunk == 0:
            W = _face_weights(a, b, c)            # [10, 13, F]
            # chunk faces: [N_FC, K, NOUT, FT]
            Wc = np.ascontiguousarray(
                W.reshape(K, NOUT, N_FC, FT).transpose(2, 0, 1, 3))
            in_maps_batch_w = Wc
        pts = body_verts[bi][chunk * P_CORE:(chunk + 1) * P_CORE]
        in_maps.append({"phi": _features(pts), "w": in_maps_batch_w})

    res = run_bass_kernel_spmd(nc, in_maps, core_ids=list(range(N_CORES)))

    losses = []
    for bi in range(B):
        md = np.concatenate([
            np.asarray(res.results[bi * (N_CORES // B) + ch]["out"])
            .T.reshape(-1)
            for ch in range(N_CORES // B)
        ])
        height = (mesh_verts[bi][:, 1].max() - mesh_verts[bi][:, 1].min())
        losses.append(math.sqrt(float(md.astype(np.float64).sum())) / float(height))
    return np.float32(sum(losses) / B)


# revision 29
# speedup vs baseline: 2.1284x; 1.4871x over previous
"""Point-cloud-to-mesh loss on 8 Trainium2 cores.

Math: the reference loss only needs, per query point, the minimum squared
distance to any mesh triangle (the residual ``p - cp`` has squared norm equal
to the min point-triangle distance^2).  For a triangle (a,b,c) with edges
ab, ac, bc the min distance^2 decomposes into four candidates:

  e_edge = E_unc + relu(max(h1, h2))^2   for each of the 3 edges, where
           E_unc = f - d^2/L  (unclamped line distance^2),
           h1 = (d - L)/sqrt(L), h2 = -d/sqrt(L)  (clamp overshoots)
  r_in   = max(f - (vb*d1 + vc*d2)/det, 0) + BIG*(relu(-va/det)+...)
           (plane distance^2, masked unless the projection is interior)

Every E_unc / h / penalty argument is an affine or quadratic polynomial in p,
so with point features phi(p) = [1,x,y,z,x^2,y^2,z^2,xy,xz,yz] all 13
quantities per (point, face) pair are a single K=10 matmul.  The TensorE
produces them; ScalarE does relu/square; VectorE does max/add/min and the
fused (min, min-reduce, running-min) via tensor_tensor_reduce.

Sharding: 8 cores = 2 batches x 4 point-chunks of 2048 points; every core
holds all 8192 faces of its batch.  Each core returns per-point min-d^2;
the host does the final sqrt / height normalization / mean.
"""
import math

import numpy as np

import concourse.bass as bass
import concourse.bacc as bacc
import concourse.mybir as mybir
import concourse.tile as tile
from concourse.bass_utils import run_bass_kernel_spmd

# Problem shape (hardcoded per spec)
B = 2
P_FULL = 8192           # query points per batch
V = 4098
F = 8192                # faces per batch
N_CORES = 8
P_CORE = (B * P_FULL) // N_CORES     # 2048 points per core
N_PT = P_CORE // 128                 # 16 point tiles
FT = 512                             # faces per chunk
N_FC = F // FT                       # 16 face chunks
K = 10                               # monomial features
NOUT = 13                            # per-face matmul outputs

EPS = 1e-12
BIGP = 1e6
ACC_INIT = 1e30

FP32 = mybir.dt.float32
AX = mybir.AxisListType
OP = mybir.AluOpType
AF = mybir.ActivationFunctionType

_NC_CACHE = None


def build_nc(repeats=1, gp_offload=True, WBUFS=2, SBUFS=4, psum_cfg=2, gp_z=0):
    """Build the SPMD Bass program (same program on all 8 cores).
    repeats>1 re-runs the whole face sweep (idempotent min) for timing."""
    nc = bacc.Bacc("TRN2")
    phi_d = nc.dram_tensor("phi", [K, P_CORE], FP32, kind="ExternalInput")
    w_d = nc.dram_tensor("w", [N_FC, K, NOUT, FT], FP32, kind="ExternalInput")
    out_d = nc.dram_tensor("out", [128, N_PT], FP32, kind="ExternalOutput")

    with tile.TileContext(nc) as tc:
        with (
            tc.tile_pool(name="persist", bufs=1) as persist,
            tc.tile_pool(name="wpool", bufs=WBUFS) as wpool,
            tc.tile_pool(name="sb", bufs=SBUFS) as sb,
            tc.tile_pool(name="psumE3",
                         bufs={0: 1, 1: 2, 2: 1, 3: 1}[psum_cfg],
                         space="PSUM") as psumE3,
            tc.tile_pool(name="psumHP",
                         bufs={0: 3, 1: 1, 2: 4, 3: 5}[psum_cfg],
                         space="PSUM") as psumHP,
            tc.tile_pool(name="psumR",
                         bufs={0: 2, 1: 1, 2: 1, 3: 1}[psum_cfg],
                         space="PSUM") as psumR,
        ):
            if psum_cfg == 3:
                psumR = psumHP  # RIN rides the HP rotation (R pool unused)
            # phi replicated at partition offsets 0/32/64/96 so matmuls can
            # cycle the 4 PE row-groups concurrently (K=10 << 128).
            phi_s = persist.tile([128, P_CORE], FP32)
            for g in range(4):
                nc.gpsimd.dma_start(out=phi_s[32 * g:32 * g + K, :],
                                    in_=phi_d[:, :])

            acc = [persist.tile([128, N_PT], FP32, tag=f"acc{i}", name=f"acc{i}")
                   for i in range(2)]
            nc.vector.memset(acc[0], ACC_INIT)

            for it in range(repeats * N_FC):
                fc = it % N_FC
                w_s = wpool.tile([128, NOUT, FT], FP32, tag="w")
                for g in range(4):
                    nc.gpsimd.dma_start(out=w_s[32 * g:32 * g + K, :, :],
                                        in_=w_d[fc])
                a_prev = acc[it % 2]
                a_cur = acc[(it + 1) % 2]
                for pt in range(N_PT):
                    mm_ctr = [0]

                    def mm(j, pool, tag, out=None):
                        if out is None:
                            out = pool.tile([128, FT], FP32, tag=tag,
                                            name=f"mm{j}")
                        g = mm_ctr[0] % 4
                        mm_ctr[0] += 1
                        o = 32 * g
                        nc.tensor.matmul(
                            out, phi_s[o:o + K, pt * 128:(pt + 1) * 128],
                            w_s[o:o + K, j, :], tile_position=(o, 0))
                        return out

                    # --- edges: E3 psum [128,3,FT]; overshoots -> ZZ3 sbuf ---
                    E3 = psumE3.tile([128, 3, FT], FP32, tag="E3", name="E3")
                    Z3 = sb.tile([128, 3, FT], FP32, tag="Z3")
                    ZZ3 = sb.tile([128, 3, FT], FP32, tag="ZZ3")
                    for i, (je, jh1, jh2) in enumerate(
                            ((0, 7, 8), (1, 9, 10), (2, 11, 12))):
                        mm(je, None, None, out=E3[:, i, :])
                        h1_ps = mm(jh1, psumHP, "HP")
                        h2_ps = mm(jh2, psumHP, "HP")
                        a2r = sb.tile([128, FT], FP32, tag="a2r")
                        nc.scalar.activation(a2r, h2_ps, AF.Relu)
                        if gp_z and i < gp_z:
                            h1c = sb.tile([128, FT], FP32, tag="h1c")
                            nc.scalar.activation(h1c, h1_ps, AF.Copy)
                            nc.gpsimd.tensor_tensor(Z3[:, i, :], h1c, a2r,
                                                    OP.max)
                        else:
                            nc.vector.tensor_tensor(Z3[:, i, :], h1_ps, a2r,
                                                    OP.max)

                    nc.scalar.activation(ZZ3, Z3, AF.Square)

                    # --- interior: pens ---
                    rin_ps = mm(3, psumR, "HP" if psum_cfg == 3 else "RIN")
                    pens = []
                    for jp in (4, 5, 6):
                        p_ps = mm(jp, psumHP, "HP")
                        pr = sb.tile([128, FT], FP32, tag=f"pen{jp}")
                        nc.scalar.activation(pr, p_ps, AF.Relu)
                        pens.append(pr)
                    pp = sb.tile([128, FT], FP32, tag="pp")
                    pp2 = sb.tile([128, FT], FP32, tag="pp2")
                    eng = nc.gpsimd if gp_offload else nc.vector
                    eng.tensor_tensor(pp, pens[0], pens[1], OP.add)
                    eng.tensor_tensor(pp2, pp, pens[2], OP.add)

                    # --- candidates tile e4 [128,4,FT]: 3 edges + interior ---
                    e4 = sb.tile([128, 4, FT], FP32, tag="e4")
                    nc.vector.tensor_tensor(e4[:, 0:3, :], E3, ZZ3, OP.add)
                    nc.vector.scalar_tensor_tensor(e4[:, 3, :], rin_ps, 0.0,
                                                   pp2, OP.max, OP.add)

                    # --- one fused min-reduce over all 4*FT candidates ---
                    red = sb.tile([128, 1], FP32, tag="red")
                    nc.vector.tensor_reduce(red, e4, AX.XY, OP.min)
                    nc.vector.tensor_tensor(a_cur[:, pt:pt + 1],
                                            a_prev[:, pt:pt + 1], red, OP.min)

            nc.sync.dma_start(out=out_d[:, :], in_=acc[(repeats * N_FC) % 2])
    nc.compile()
    return nc


def get_nc():
    global _NC_CACHE
    if _NC_CACHE is None:
        _NC_CACHE = build_nc()
    return _NC_CACHE


# ---------------- host-side weight/feature prep ----------------

def _face_weights(a, b, c):
    """[10, 13, F] fp32 monomial weights for all candidate quantities.
    Monomial basis: [1, x, y, z, x^2, y^2, z^2, xy, xz, yz]."""
    a = a.astype(np.float64)
    b = b.astype(np.float64)
    c = c.astype(np.float64)
    ab, ac, bc = b - a, c - a, c - b
    aa = (ab * ab).sum(-1)
    bb = (ab * ac).sum(-1)
    cc = (ac * ac).sum(-1)
    ll = (bc * bc).sum(-1)
    det = aa * cc - bb * bb
    inv_aa = 1.0 / np.maximum(aa, EPS)
    inv_cc = 1.0 / np.maximum(cc, EPS)
    inv_ll = 1.0 / np.maximum(ll, EPS)
    inv_det = 1.0 / np.maximum(det, EPS)
    nF = a.shape[0]

    def affine(const, lin):
        w = np.zeros((nF, 10))
        w[:, 0] = const
        w[:, 1:4] = lin
        return w

    def prod(c1, l1, c2, l2):
        w = np.zeros((nF, 10))
        w[:, 0] = c1 * c2
        w[:, 1:4] = c1[:, None] * l2 + c2[:, None] * l1
        w[:, 4] = l1[:, 0] * l2[:, 0]
        w[:, 5] = l1[:, 1] * l2[:, 1]
        w[:, 6] = l1[:, 2] * l2[:, 2]
        w[:, 7] = l1[:, 0] * l2[:, 1] + l1[:, 1] * l2[:, 0]
        w[:, 8] = l1[:, 0] * l2[:, 2] + l1[:, 2] * l2[:, 0]
        w[:, 9] = l1[:, 1] * l2[:, 2] + l1[:, 2] * l2[:, 1]
        return w

    def sqdist(v):
        w = np.zeros((nF, 10))
        w[:, 0] = (v * v).sum(-1)
        w[:, 1:4] = -2.0 * v
        w[:, 4:7] = 1.0
        return w

    d1c, d1l = -(ab * a).sum(-1), ab
    d2c, d2l = -(ac * a).sum(-1), ac
    uc, ul = -(bc * b).sum(-1), bc
    f_w = sqdist(a)
    rb_w = sqdist(b)
    vbc = d1c * cc - d2c * bb
    vbl = d1l * cc[:, None] - d2l * bb[:, None]
    vcc = d2c * aa - d1c * bb
    vcl = d2l * aa[:, None] - d1l * bb[:, None]
    vac = det - vbc - vcc
    val = -vbl - vcl
    r_aa, r_cc, r_ll = np.sqrt(inv_aa), np.sqrt(inv_cc), np.sqrt(inv_ll)

    W = np.zeros((10, NOUT, nF))
    W[:, 0] = (f_w - prod(d1c, d1l, d1c, d1l) * inv_aa[:, None]).T
    W[:, 1] = (f_w - prod(d2c, d2l, d2c, d2l) * inv_cc[:, None]).T
    W[:, 2] = (rb_w - prod(uc, ul, uc, ul) * inv_ll[:, None]).T
    W[:, 3] = (f_w - (prod(vbc, vbl, d1c, d1l)
                      + prod(vcc, vcl, d2c, d2l)) * inv_det[:, None]).T
    W[:, 4] = (affine(vac, val) * (-BIGP * inv_det)[:, None]).T
    W[:, 5] = (affine(vbc, vbl) * (-BIGP * inv_det)[:, None]).T
    W[:, 6] = (affine(vcc, vcl) * (-BIGP * inv_det)[:, None]).T
    W[:, 7] = (affine(d1c - aa, d1l) * r_aa[:, None]).T
    W[:, 8] = (affine(d1c, d1l) * (-r_aa)[:, None]).T
    W[:, 9] = (affine(d2c - cc, d2l) * r_cc[:, None]).T
    W[:, 10] = (affine(d2c, d2l) * (-r_cc)[:, None]).T
    W[:, 11] = (affine(uc - ll, ul) * r_ll[:, None]).T
    W[:, 12] = (affine(uc, ul) * (-r_ll)[:, None]).T
    return np.ascontiguousarray(W.astype(np.float32))


def _features(points):
    p = points.astype(np.float32)
    x, y, z = p[:, 0], p[:, 1], p[:, 2]
    return np.ascontiguousarray(np.stack(
        [np.ones_like(x), x, y, z, x * x, y * y, z * z, x * y, x * z, y * z],
        axis=0))


def kernel(body_verts, mesh_verts, faces):
    body_verts = np.asarray(body_verts, dtype=np.float32)
    mesh_verts = np.asarray(mesh_verts, dtype=np.float32)
    faces = np.asarray(faces, dtype=np.int32)

    nc = get_nc()

    in_maps = []
    for core in range(N_CORES):
        bi = core // (N_CORES // B)
        chunk = core % (N_CORES // B)
        a = mesh_verts[bi][faces[bi][:, 0]]
        b = mesh_verts[bi][faces[bi][:, 1]]
        c = mesh_verts[bi][faces[bi][:, 2]]
        if chunk == 0:
            W = _face_weights(a, b, c)            # [10, 13, F]
            # chunk faces: [N_FC, K, NOUT, FT]
            Wc = np.ascontiguousarray(
                W.reshape(K, NOUT, N_FC, FT).transpose(2, 0, 1, 3))
            in_maps_batch_w = Wc
        pts = body_verts[bi][chunk * P_CORE:(chunk + 1) * P_CORE]
        in_maps.append({"phi": _features(pts), "w": in_maps_batch_w})

    res = run_bass_kernel_spmd(nc, in_maps, core_ids=list(range(N_CORES)))

    losses = []
    for bi in range(B):
        md = np.concatenate([
            np.asarray(res.results[bi * (N_CORES // B) + ch]["out"])
            .T.reshape(-1)
            for ch in range(N_CORES // B)
        ])
        height = (mesh_verts[bi][:, 1].max() - mesh_verts[bi][:, 1].min())
        losses.append(math.sqrt(float(md.astype(np.float64).sum())) / float(height))
    return np.float32(sum(losses) / B)
